# revision 10
# baseline (speedup 1.0000x reference)
"""Trainium2 Bass kernel for masked two-template sparse attention.

Model (per sample, fp32 reference):
    qkv = (x @ W_qkv.T) * mask          mask: temp_mask on first 64 tokens, 1 elsewhere
    q,k,v split into 12 heads x 64
    template tokens (first 128) attend to template tokens only
    search tokens (last 324) attend to all 452 tokens
    out = concat(attn outputs) @ W_proj.T + b_proj

Sharding: data-parallel over batch, 32 samples -> 4 per NeuronCore x 8 cores.
All attention math is done in "transposed" layout (channels on partitions):
    x^T (PE transpose) -> q^T,k^T = Wqkv^T.T @ x^T ; v natural = x^T.T @ Wv^T
    S^T = k^T.T @ q^T  (row-tiled pairs of 64-wide heads run concurrently)
    E^T = exp(S^T * scale)             (no max subtraction; |S| <~ 6)
    [attn^T_unnorm ; sums] = [v_h|ones].T @ E^T  (one matmul per head/k-chunk;
                              partitions 0:64 = attn.V, 64:128 = denominator)
    attn^T = attn^T_unnorm * recip(sums)         (partition-shifted DVE ops)
    y = attn^T.T @ Wp^T                (+ bias added on host)

Emission keeps natural per-sample order (front then back); the Tile
list-scheduler prefers earlier-emitted ready work, so the current
sample's ACT-bound attention keeps priority while its stalls are
filled by the next sample's front-end projection matmuls.

x^T is produced by an SBUF->SBUF X-bar DMA transpose of the bf16 cast
(no PE/PSUM involvement). Template scores ride in the same matmul as
search scores for k-chunk 0 (same stationary k^T tile, all 452 queries
streamed), so exp consumes [Et|E0] and E1 in one 904-col activation.

PSUM (8 banks): tag "sc" (scores) [128,1024]f32 x2 = 4 banks, tag "pv"
(PV accum + denominators) [128,512]f32 x2 = 2 banks, tag "mm"
(q/k chunks, v halves, proj halves) [128,512]f32 x2 = 2 banks. The
score stream, PV stream and projection stream must not share slots or
the ACT-gated attention rotation starves the projection fill.
"""

import numpy as np
import ml_dtypes

import concourse.bass as bass
import concourse.mybir as mybir
import concourse.tile as tile
from concourse.bass_utils import run_bass_kernel_spmd
from concourse.masks import make_identity

# ---------------- configuration ----------------
PROJ_DT_NAME = "bfloat16"
ATT_DT_NAME = "bfloat16"
TRACE = False        # request NTFF profile on run
PHASES = 6           # kept for test.py compatibility (unused)
REPS = 1             # timing: repeat the whole computation inside the NEFF

NCORES = 8
S = 4                # samples per core
N, C, H, DH = 452, 768, 12, 64
NMT, NS = 128, 324   # template tokens / search tokens
SCALE = DH ** -0.5
TCH = [(0, 128), (128, 256), (256, 384), (384, 452)]  # token chunks
KC = 6               # channel chunks of 128

_F32 = mybir.dt.float32


def _legalize_waits(nc, max_waits=1):
    """This container's walrus accepts at most one sync-wait per instruction;
    hoist extra waits onto dedicated NOPs in front of the instruction."""
    n_split = 0
    for f in nc.m.functions:
        for bb in f.blocks:
            new_insts = []
            for inst in bb.instructions:
                si = inst.sync_info
                if si is not None and si.on_wait and len(si.on_wait) > max_waits:
                    waits = list(si.on_wait)
                    for i, w in enumerate(waits[:-max_waits]):
                        new_insts.append(
                            mybir.InstNoOp(
                                name=f"{inst.name}-w{i}",
                                sync_info=mybir.SyncInfo(on_wait=[w], on_update=[]),
                                bass_nofuse=True,
                                engine=inst.engine,
                            )
                        )
                    si.on_wait = waits[-max_waits:]
                    n_split += 1
                new_insts.append(inst)
            bb.instructions = new_insts
    return n_split


def build_module():
    pdt = getattr(mybir.dt, PROJ_DT_NAME)
    adt = getattr(mybir.dt, ATT_DT_NAME)

    nc = bass.Bass("TRN2", target_bir_lowering=False, debug=False)
    x_d = nc.dram_tensor("x", [S, N, C], _F32, kind="ExternalInput").ap()
    m_d = nc.dram_tensor("tmask", [S, 64], _F32, kind="ExternalInput").ap()
    wq_d = nc.dram_tensor("wqkvT", [C, 3 * C], pdt, kind="ExternalInput").ap()
    wp_d = nc.dram_tensor("wpT", [C, C], pdt, kind="ExternalInput").ap()
    y_d = nc.dram_tensor("y", [S, N, C], _F32, kind="ExternalOutput").ap()

    Exp = mybir.ActivationFunctionType.Exp

    with tile.TileContext(nc) as tc:
        with (
            tc.tile_pool(name="const", bufs=1) as cp,
            tc.tile_pool(name="work", bufs=1) as wk,
            tc.tile_pool(name="ps", bufs=2, space="PSUM") as pp,
        ):
            # ---- persistent constants ----
            wq_sb = []
            for i in range(KC):
                w = cp.tile([128, 3 * C], pdt, name=f"wq{i}", tag=f"wq{i}")
                nc.scalar.dma_start(w[:, :], wq_d[i * 128:(i + 1) * 128, :])
                wq_sb.append(w)
            wp_sb = []
            for i in range(KC):
                w = cp.tile([128, C], pdt, name=f"wp{i}", tag=f"wp{i}")
                nc.scalar.dma_start(w[:, :], wp_d[i * 128:(i + 1) * 128, :])
                wp_sb.append(w)
            ident = cp.tile([128, 128], adt, name="ident", tag="ident")
            make_identity(nc, ident)

            # persistent [v_h | ones] tiles, 2-deep ping-pong across samples;
            # the ones half (softmax denominator trick) is memset exactly once
            vt_bufs = []
            for b in range(2):
                row = []
                for ti in range(4):
                    t = cp.tile(
                        [128, H * 128], adt, name=f"vtc{b}{ti}", tag=f"vtc{b}{ti}"
                    )
                    th = t.rearrange("p (h c) -> p h c", c=128)
                    nc.gpsimd.memset(th[:, :, DH:128], 1.0)
                    row.append(t)
                vt_bufs.append(row)

            samples = [si for _rep in range(REPS) for si in range(S)]

            def front(i):
                """load + mask + cast + x^T + q/k/v projections of sample i"""
                s = samples[i]
                xn = wk.tile([128, 4, C], _F32, name="xn", tag="xn", bufs=2)
                nc.sync.dma_start(
                    xn[:, 0:3, :],
                    x_d[s, 0:384, :].rearrange("(c p) d -> p c d", p=128),
                )
                nc.sync.dma_start(xn[0:68, 3, :], x_d[s, 384:452, :])
                msk = wk.tile([64, 1], _F32, name="msk", tag="msk", bufs=2)
                nc.sync.dma_start(msk[:, :], m_d[s, :].unsqueeze(1))
                nc.vector.tensor_scalar_mul(xn[0:64, 0, :], xn[0:64, 0, :], msk[0:64, :])
                # cast to bf16 on GpSimd (otherwise idle) so PE transposes
                # run at 1 cycle/row and DVE stays free for PSUM drains
                xnc = wk.tile([128, 4, C], adt, name="xnc", tag="xnc", bufs=2)
                nc.gpsimd.tensor_copy(xnc[:, :, :], xn[:, :, :])

                # ---- x^T via PE transpose (staged through the pv tag;
                # its 1-bank slots are idle during the front phase) ----
                xTb = wk.tile([128, KC, N], pdt, name="xTb", tag="xTb", bufs=2)
                xT = [xTb[:, cc, :] for cc in range(KC)]
                for ti, (t0, t1) in enumerate(TCH):
                    tsz = t1 - t0
                    ptr = pp.tile([128, 1024], adt, name="ptr", tag="pv")
                    for cc in range(KC):
                        nc.tensor.transpose(
                            ptr[:, cc * 128: cc * 128 + tsz],
                            xnc[0:tsz, ti, cc * 128:(cc + 1) * 128],
                            ident[0:tsz, 0:tsz],
                        )
                    nc.vector.tensor_copy(
                        xTb[:, :, t0:t1],
                        ptr.rearrange("p (c k) -> p c k", k=128)[:, 0:KC, 0:tsz],
                    )

                # ---- q^T / k^T projections (1-bank PSUM chunks) ----
                qk = []
                for p in range(6):
                    t = wk.tile([128, 2, N], adt, name=f"qk{p}", tag=f"qk{p}", bufs=2)
                    for b, base in ((0, p * 128), (1, C + p * 128)):
                        pq = pp.tile([128, 512], _F32, name="pq", tag="mm")
                        for kc in range(KC):
                            nc.tensor.matmul(
                                pq[:, 0:N],
                                wq_sb[kc][:, base:base + 128],
                                xT[kc],
                                start=(kc == 0),
                                stop=(kc == KC - 1),
                            )
                        nc.vector.tensor_copy(t[:, b, :], pq[:, 0:N])
                    qk.append(t)

                # ---- v (natural layout) into persistent [v|ones] tiles,
                # two 1-bank PSUM halves (heads 0:8 / 8:12) ----
                vt = vt_bufs[i % 2]
                for ti, (t0, t1) in enumerate(TCH):
                    tsz = t1 - t0
                    th = vt[ti].rearrange("p (h c) -> p h c", c=128)
                    for w0, w1, h0, h1 in ((1536, 2048, 0, 8), (2048, 2304, 8, 12)):
                        pv = pp.tile([128, 512], _F32, name="pvproj", tag="mm")
                        for kc in range(KC):
                            nc.tensor.matmul(
                                pv[0:tsz, 0:w1 - w0],
                                xT[kc][:, t0:t1],
                                wq_sb[kc][:, w0:w1],
                                start=(kc == 0),
                                stop=(kc == KC - 1),
                            )
                        nc.vector.tensor_copy(
                            th[0:tsz, h0:h1, 0:DH],
                            pv[0:tsz, 0:(h1 - h0) * DH].rearrange(
                                "p (h c) -> p h c", c=DH
                            ),
                        )
                return {"qk": qk, "vt": vt}

            def back(i, st):
                """attention + output projection of sample i"""
                s = samples[i]
                qk, vt = st["qk"], st["vt"]
                attnT = []
                for p in range(6):
                    qc = qk[p][:, 0, :]
                    kt = qk[p][:, 1, :]
                    # scores + exp, per head. k-chunk 0 streams ALL 452
                    # queries (template queries see only template keys =
                    # chunk 0), so template scores ride in the same matmul
                    # and the same 904-col exp as the chunk-0/1 search
                    # scores: es = [Et | E0 | E1 | junk | E2 | E3].
                    es_pair = []
                    for hh in range(2):
                        b0 = hh * 64
                        es = wk.tile([128, 1552], adt, name="es", tag="es", bufs=3)
                        # half 0: chunk0 (all queries) + chunk1 (search)
                        ps = pp.tile([128, 1024], _F32, name="ps", tag="sc")
                        nc.tensor.matmul(
                            ps[0:128, 0:N],
                            kt[b0:b0 + 64, 0:128],
                            qc[b0:b0 + 64, 0:N],
                            start=True, stop=True,
                            tile_position=(b0, 0),
                            skip_group_check=True,
                        )
                        nc.tensor.matmul(
                            ps[0:128, 512:512 + NS],
                            kt[b0:b0 + 64, 128:256],
                            qc[b0:b0 + 64, NMT:N],
                            start=True, stop=True,
                            tile_position=(b0, 0),
                            skip_group_check=True,
                        )
                        nc.scalar.activation(
                            es[:, 0:2 * N].rearrange("p (b k) -> p b k", k=N),
                            ps.rearrange("p (b k) -> p b k", b=2)[:, :, 0:N],
                            Exp,
                            scale=SCALE,
                        )
                        # half 1: chunks 2,3 (search queries only)
                        ps = pp.tile([128, 1024], _F32, name="ps", tag="sc")
                        for j in range(2):
                            k0, k1 = TCH[2 + j]
                            nc.tensor.matmul(
                                ps[0:k1 - k0, j * 512: j * 512 + NS],
                                kt[b0:b0 + 64, k0:k1],
                                qc[b0:b0 + 64, NMT:N],
                                start=True, stop=True,
                                tile_position=(b0, 0),
                                skip_group_check=True,
                            )
                        nc.scalar.activation(
                            es[:, 2 * N:2 * N + 2 * NS].rearrange(
                                "p (b k) -> p b k", k=NS
                            ),
                            ps.rearrange("p (b k) -> p b k", b=2)[:, :, 0:NS],
                            Exp,
                            scale=SCALE,
                        )
                        es_pair.append(es)

                    # PV + denominators in one matmul per (head, k-chunk):
                    # out partitions 0:64 = attn^T unnorm, 64:128 = sums bcast
                    EOFF = [NMT, N, 2 * N, 2 * N + NS]  # E0..E3 col offsets
                    at = wk.tile([128, N], pdt, name=f"attnT{p}", tag=f"attnT{p}", bufs=2)
                    for hh in range(2):
                        h = 2 * p + hh
                        es = es_pair[hh]
                        pvps = pp.tile([128, 512], _F32, name="pvps", tag="pv")
                        # chunk 0 carries template AND search queries in one
                        # 452-col matmul (rhs = [Et|E0], same stationary);
                        # chunks 1-3 accumulate into the search cols only
                        nc.tensor.matmul(
                            pvps[:, 0:N],
                            vt[0][0:NMT, h * 128:(h + 1) * 128],
                            es[0:NMT, 0:N],
                            start=True, stop=False,
                            skip_group_check=True,
                        )
                        for kcj in range(1, 4):
                            k0, k1 = TCH[kcj]
                            ksz = k1 - k0
                            nc.tensor.matmul(
                                pvps[:, NMT:N],
                                vt[kcj][0:ksz, h * 128:(h + 1) * 128],
                                es[0:ksz, EOFF[kcj]:EOFF[kcj] + NS],
                                start=False, stop=(kcj == 3),
                                skip_group_check=True,
                            )
                        r = wk.tile([64, N], _F32, name="r", tag="r", bufs=3)
                        nc.vector.reciprocal(r[0:64, :], pvps[64:128, 0:N])
                        nc.vector.tensor_mul(
                            at[hh * 64:(hh + 1) * 64, :], pvps[0:64, 0:N], r[0:64, :]
                        )
                    attnT.append(at)

                # ---- output projection (bias added on host) ----
                for (q0, q1) in TCH:
                    qsz = q1 - q0
                    ysb = wk.tile([128, C], _F32, name="ysb", tag="ysb", bufs=3)
                    for w0, w1 in ((0, 512), (512, 768)):
                        py = pp.tile([128, 512], _F32, name="py", tag="mm")
                        for mc in range(KC):
                            nc.tensor.matmul(
                                py[0:qsz, 0:w1 - w0],
                                attnT[mc][:, q0:q1],
                                wp_sb[mc][:, w0:w1],
                                start=(mc == 0), stop=(mc == KC - 1),
                            )
                        nc.scalar.copy(ysb[0:qsz, w0:w1], py[0:qsz, 0:w1 - w0])
                    nc.sync.dma_start(y_d[s, q0:q1, :], ysb[0:qsz, :])

            # Software pipeline with decoupled priorities: PSUM slot
            # rotation binds in tile-creation order, engine preference in
            # bass_priority order. Emit front(i+1) BEFORE back(i) so the
            # next sample's projections get PSUM slots during attention,
            # but number back(i) earlier so the scheduler still prefers
            # the critical attention stream when both are ready.
            STRIDE = 4000
            tc.cur_priority = 500
            prev = front(0)
            for i in range(len(samples)):
                if i + 1 < len(samples):
                    tc.cur_priority = STRIDE * i + STRIDE // 2
                    nxt = front(i + 1)
                else:
                    nxt = None
                tc.cur_priority = STRIDE * i + 1000
                back(i, prev)
                prev = nxt
            tc.cur_priority = STRIDE * len(samples)

    _legalize_waits(nc)
    return nc


_NC_CACHE = {}


def _get_module():
    key = (PROJ_DT_NAME, ATT_DT_NAME, PHASES, REPS)
    if key not in _NC_CACHE:
        _NC_CACHE[key] = build_module()
    return _NC_CACHE[key]


def kernel(x, temp_mask, W_qkv, W_proj, b_proj, t_h=None, t_w=None, s_h=None, s_w=None):
    x = np.asarray(x, dtype=np.float32)
    temp_mask = np.asarray(temp_mask, dtype=np.float32)
    B = x.shape[0]
    assert x.shape == (32, N, C), x.shape

    pdt_np = ml_dtypes.bfloat16 if PROJ_DT_NAME == "bfloat16" else np.float32
    wqkvT = np.ascontiguousarray(np.asarray(W_qkv, np.float32).T).astype(pdt_np)
    wpT = np.ascontiguousarray(np.asarray(W_proj, np.float32).T).astype(pdt_np)
    tm = np.ascontiguousarray(temp_mask.reshape(B, 64))

    nc = _get_module()
    per = B // NCORES
    in_maps = [
        {
            "x": np.ascontiguousarray(x[c * per:(c + 1) * per]),
            "tmask": np.ascontiguousarray(tm[c * per:(c + 1) * per]),
            "wqkvT": wqkvT,
            "wpT": wpT,
        }
        for c in range(NCORES)
    ]
    res = run_bass_kernel_spmd(nc, in_maps, core_ids=list(range(NCORES)), trace=TRACE)
    kernel.last_result = res
    y = np.concatenate([res.results[c]["y"] for c in range(NCORES)], axis=0)
    y = y + np.asarray(b_proj, np.float32)[None, None, :]
    return y.astype(np.float32)


# revision 12
# speedup vs baseline: 1.1053x; 1.1053x over previous
"""Trainium2 Bass kernel for masked two-template sparse attention.

Model (per sample, fp32 reference):
    qkv = (x @ W_qkv.T) * mask          mask: temp_mask on first 64 tokens, 1 elsewhere
    q,k,v split into 12 heads x 64
    template tokens (first 128) attend to template tokens only
    search tokens (last 324) attend to all 452 tokens
    out = concat(attn outputs) @ W_proj.T + b_proj

Sharding: data-parallel over batch, 32 samples -> 4 per NeuronCore x 8 cores.
All attention math is done in "transposed" layout (channels on partitions):
    x^T (PE transpose) -> q^T,k^T = Wqkv^T.T @ x^T ; v natural = x^T.T @ Wv^T
    S^T = k^T.T @ q^T  (row-tiled pairs of 64-wide heads run concurrently)
    E^T = exp(S^T * scale)             (no max subtraction; |S| <~ 6)
    [attn^T_unnorm ; sums] = [v_h|ones].T @ E^T  (one matmul per head/k-chunk;
                              partitions 0:64 = attn.V, 64:128 = denominator)
    attn^T = attn^T_unnorm * recip(sums)         (partition-shifted DVE ops)
    y = attn^T.T @ Wp^T                (+ bias added on host)

Emission keeps natural per-sample order (front then back); the Tile
list-scheduler prefers earlier-emitted ready work, so the current
sample's ACT-bound attention keeps priority while its stalls are
filled by the next sample's front-end projection matmuls.

x^T is produced by an SBUF->SBUF X-bar DMA transpose of the bf16 cast
(no PE/PSUM involvement). Template scores ride in the same matmul as
search scores for k-chunk 0 (same stationary k^T tile, all 452 queries
streamed), so exp consumes [Et|E0] and E1 in one 904-col activation.

PSUM (8 banks): tag "sc" (scores) [128,1024]f32 x2 = 4 banks, tag "pv"
(PV accum + denominators) [128,512]f32 x2 = 2 banks, tag "mm"
(q/k chunks, v halves, proj halves) [128,512]f32 x2 = 2 banks. The
score stream, PV stream and projection stream must not share slots or
the ACT-gated attention rotation starves the projection fill.
"""

import numpy as np
import ml_dtypes

import concourse.bass as bass
import concourse.mybir as mybir
import concourse.tile as tile
from concourse.bass_utils import run_bass_kernel_spmd
from concourse.masks import make_identity

# ---------------- configuration ----------------
PROJ_DT_NAME = "bfloat16"
ATT_DT_NAME = "bfloat16"
TRACE = False        # request NTFF profile on run
PHASES = 6           # kept for test.py compatibility (unused)
REPS = 1             # timing: repeat the whole computation inside the NEFF

NCORES = 8
S = 4                # samples per core
N, C, H, DH = 452, 768, 12, 64
NMT, NS = 128, 324   # template tokens / search tokens
SCALE = DH ** -0.5
TCH = [(0, 128), (128, 256), (256, 384), (384, 452)]  # token chunks
KC = 6               # channel chunks of 128

_F32 = mybir.dt.float32


def _legalize_waits(nc, max_waits=1):
    """This container's walrus accepts at most one sync-wait per instruction;
    hoist extra waits onto dedicated NOPs in front of the instruction."""
    n_split = 0
    for f in nc.m.functions:
        for bb in f.blocks:
            new_insts = []
            for inst in bb.instructions:
                si = inst.sync_info
                if si is not None and si.on_wait and len(si.on_wait) > max_waits:
                    waits = list(si.on_wait)
                    for i, w in enumerate(waits[:-max_waits]):
                        new_insts.append(
                            mybir.InstNoOp(
                                name=f"{inst.name}-w{i}",
                                sync_info=mybir.SyncInfo(on_wait=[w], on_update=[]),
                                bass_nofuse=True,
                                engine=inst.engine,
                            )
                        )
                    si.on_wait = waits[-max_waits:]
                    n_split += 1
                new_insts.append(inst)
            bb.instructions = new_insts
    return n_split


def build_module():
    pdt = getattr(mybir.dt, PROJ_DT_NAME)
    adt = getattr(mybir.dt, ATT_DT_NAME)

    nc = bass.Bass("TRN2", target_bir_lowering=False, debug=False)
    x_d = nc.dram_tensor("x", [S, N, C], _F32, kind="ExternalInput").ap()
    m_d = nc.dram_tensor("tmask", [S, 64], _F32, kind="ExternalInput").ap()
    wq_d = nc.dram_tensor("wqkvT", [C, 3 * C], pdt, kind="ExternalInput").ap()
    wp_d = nc.dram_tensor("wpT", [C, C], pdt, kind="ExternalInput").ap()
    y_d = nc.dram_tensor("y", [S, N, C], _F32, kind="ExternalOutput").ap()

    Exp = mybir.ActivationFunctionType.Exp

    with tile.TileContext(nc) as tc:
        with (
            tc.tile_pool(name="const", bufs=1) as cp,
            tc.tile_pool(name="work", bufs=1) as wk,
            tc.tile_pool(name="ps", bufs=2, space="PSUM") as pp,
        ):
            # ---- persistent constants ----
            wq_sb = []
            for i in range(KC):
                w = cp.tile([128, 3 * C], pdt, name=f"wq{i}", tag=f"wq{i}")
                nc.scalar.dma_start(w[:, :], wq_d[i * 128:(i + 1) * 128, :])
                wq_sb.append(w)
            wp_sb = []
            for i in range(KC):
                w = cp.tile([128, C], pdt, name=f"wp{i}", tag=f"wp{i}")
                nc.scalar.dma_start(w[:, :], wp_d[i * 128:(i + 1) * 128, :])
                wp_sb.append(w)
            ident = cp.tile([128, 128], adt, name="ident", tag="ident")
            make_identity(nc, ident)

            # persistent [v_h | ones] tiles, 2-deep ping-pong across samples;
            # the ones half (softmax denominator trick) is memset exactly once
            vt_bufs = []
            for b in range(2):
                row = []
                for ti in range(4):
                    t = cp.tile(
                        [128, H * 128], adt, name=f"vtc{b}{ti}", tag=f"vtc{b}{ti}"
                    )
                    th = t.rearrange("p (h c) -> p h c", c=128)
                    nc.gpsimd.memset(th[:, :, DH:128], 1.0)
                    row.append(t)
                vt_bufs.append(row)

            samples = [si for _rep in range(REPS) for si in range(S)]

            def front(i):
                """load + mask + cast + x^T + q/k/v projections of sample i"""
                s = samples[i]
                xn = wk.tile([128, 4, C], _F32, name="xn", tag="xn", bufs=2)
                nc.sync.dma_start(
                    xn[:, 0:3, :],
                    x_d[s, 0:384, :].rearrange("(c p) d -> p c d", p=128),
                )
                nc.sync.dma_start(xn[0:68, 3, :], x_d[s, 384:452, :])
                msk = wk.tile([64, 1], _F32, name="msk", tag="msk", bufs=2)
                nc.sync.dma_start(msk[:, :], m_d[s, :].unsqueeze(1))
                nc.vector.tensor_scalar_mul(xn[0:64, 0, :], xn[0:64, 0, :], msk[0:64, :])
                # cast to bf16 on GpSimd (otherwise idle) so PE transposes
                # run at 1 cycle/row and DVE stays free for PSUM drains
                xnc = wk.tile([128, 4, C], adt, name="xnc", tag="xnc", bufs=2)
                nc.gpsimd.tensor_copy(xnc[:, :, :], xn[:, :, :])

                # ---- x^T via X-bar DMA transpose (SBUF->SBUF, bf16):
                # standard DMACopy descriptors with the xbar, keeps the
                # 96 PE transpose matmuls (and their serial LDWEIGHTS)
                # off the tensor engine ----
                # (free dim padded to 464: the X-bar needs source rows
                # divisible by 16, so the last 68-token chunk moves as 80
                # rows; cols 452:464 are junk and never read)
                xTb = wk.tile([128, KC, 464], pdt, name="xTb", tag="xTb", bufs=2)
                xT = [xTb[:, cc, 0:N] for cc in range(KC)]
                for ti, (t0, t1) in enumerate(TCH):
                    tsz = -(-(t1 - t0) // 16) * 16
                    nc.sync.dma_start(
                        xTb[:, :, t0:t0 + tsz],
                        xnc[0:tsz, ti, :],
                        transpose=True,
                    )

                # ---- q^T / k^T projections (1-bank PSUM chunks) ----
                qk = []
                for p in range(6):
                    t = wk.tile([128, 2, N], adt, name=f"qk{p}", tag=f"qk{p}", bufs=2)
                    for b, base in ((0, p * 128), (1, C + p * 128)):
                        pq = pp.tile([128, 512], _F32, name="pq", tag="mm")
                        for kc in range(KC):
                            nc.tensor.matmul(
                                pq[:, 0:N],
                                wq_sb[kc][:, base:base + 128],
                                xT[kc],
                                start=(kc == 0),
                                stop=(kc == KC - 1),
                            )
                        nc.vector.tensor_copy(t[:, b, :], pq[:, 0:N])
                    qk.append(t)

                # ---- v (natural layout) into persistent [v|ones] tiles,
                # two 1-bank PSUM halves (heads 0:8 / 8:12) ----
                vt = vt_bufs[i % 2]
                for ti, (t0, t1) in enumerate(TCH):
                    tsz = t1 - t0
                    th = vt[ti].rearrange("p (h c) -> p h c", c=128)
                    for w0, w1, h0, h1 in ((1536, 2048, 0, 8), (2048, 2304, 8, 12)):
                        pv = pp.tile([128, 512], _F32, name="pvproj", tag="mm")
                        for kc in range(KC):
                            nc.tensor.matmul(
                                pv[0:tsz, 0:w1 - w0],
                                xT[kc][:, t0:t1],
                                wq_sb[kc][:, w0:w1],
                                start=(kc == 0),
                                stop=(kc == KC - 1),
                            )
                        nc.vector.tensor_copy(
                            th[0:tsz, h0:h1, 0:DH],
                            pv[0:tsz, 0:(h1 - h0) * DH].rearrange(
                                "p (h c) -> p h c", c=DH
                            ),
                        )
                return {"qk": qk, "vt": vt}

            def back(i, st):
                """attention + output projection of sample i"""
                s = samples[i]
                qk, vt = st["qk"], st["vt"]
                attnT = []
                for p in range(6):
                    qc = qk[p][:, 0, :]
                    kt = qk[p][:, 1, :]
                    # scores + exp, per head. k-chunk 0 streams ALL 452
                    # queries (template queries see only template keys =
                    # chunk 0), so template scores ride in the same matmul
                    # and the same 904-col exp as the chunk-0/1 search
                    # scores: es = [Et | E0 | E1 | junk | E2 | E3].
                    es_pair = []
                    for hh in range(2):
                        b0 = hh * 64
                        es = wk.tile([128, 1552], adt, name="es", tag="es", bufs=3)
                        # half 0: chunk0 (all queries) + chunk1 (search)
                        ps = pp.tile([128, 1024], _F32, name="ps", tag="sc")
                        nc.tensor.matmul(
                            ps[0:128, 0:N],
                            kt[b0:b0 + 64, 0:128],
                            qc[b0:b0 + 64, 0:N],
                            start=True, stop=True,
                            tile_position=(b0, 0),
                            skip_group_check=True,
                        )
                        nc.tensor.matmul(
                            ps[0:128, 512:512 + NS],
                            kt[b0:b0 + 64, 128:256],
                            qc[b0:b0 + 64, NMT:N],
                            start=True, stop=True,
                            tile_position=(b0, 0),
                            skip_group_check=True,
                        )
                        nc.scalar.activation(
                            es[:, 0:2 * N].rearrange("p (b k) -> p b k", k=N),
                            ps.rearrange("p (b k) -> p b k", b=2)[:, :, 0:N],
                            Exp,
                            scale=SCALE,
                        )
                        # half 1: chunks 2,3 (search queries only)
                        ps = pp.tile([128, 1024], _F32, name="ps", tag="sc")
                        for j in range(2):
                            k0, k1 = TCH[2 + j]
                            nc.tensor.matmul(
                                ps[0:k1 - k0, j * 512: j * 512 + NS],
                                kt[b0:b0 + 64, k0:k1],
                                qc[b0:b0 + 64, NMT:N],
                                start=True, stop=True,
                                tile_position=(b0, 0),
                                skip_group_check=True,
                            )
                        nc.scalar.activation(
                            es[:, 2 * N:2 * N + 2 * NS].rearrange(
                                "p (b k) -> p b k", k=NS
                            ),
                            ps.rearrange("p (b k) -> p b k", b=2)[:, :, 0:NS],
                            Exp,
                            scale=SCALE,
                        )
                        es_pair.append(es)

                    # PV + denominators in one matmul per (head, k-chunk):
                    # out partitions 0:64 = attn^T unnorm, 64:128 = sums bcast
                    EOFF = [NMT, N, 2 * N, 2 * N + NS]  # E0..E3 col offsets
                    at = wk.tile([128, N], pdt, name=f"attnT{p}", tag=f"attnT{p}", bufs=2)
                    for hh in range(2):
                        h = 2 * p + hh
                        es = es_pair[hh]
                        pvps = pp.tile([128, 512], _F32, name="pvps", tag="pv")
                        # chunk 0 carries template AND search queries in one
                        # 452-col matmul (rhs = [Et|E0], same stationary);
                        # chunks 1-3 accumulate into the search cols only
                        nc.tensor.matmul(
                            pvps[:, 0:N],
                            vt[0][0:NMT, h * 128:(h + 1) * 128],
                            es[0:NMT, 0:N],
                            start=True, stop=False,
                            skip_group_check=True,
                        )
                        for kcj in range(1, 4):
                            k0, k1 = TCH[kcj]
                            ksz = k1 - k0
                            nc.tensor.matmul(
                                pvps[:, NMT:N],
                                vt[kcj][0:ksz, h * 128:(h + 1) * 128],
                                es[0:ksz, EOFF[kcj]:EOFF[kcj] + NS],
                                start=False, stop=(kcj == 3),
                                skip_group_check=True,
                            )
                        r = wk.tile([64, N], _F32, name="r", tag="r", bufs=3)
                        nc.vector.reciprocal(r[0:64, :], pvps[64:128, 0:N])
                        nc.vector.tensor_mul(
                            at[hh * 64:(hh + 1) * 64, :], pvps[0:64, 0:N], r[0:64, :]
                        )
                    attnT.append(at)

                # ---- output projection (bias added on host) ----
                for (q0, q1) in TCH:
                    qsz = q1 - q0
                    ysb = wk.tile([128, C], _F32, name="ysb", tag="ysb", bufs=3)
                    for w0, w1 in ((0, 512), (512, 768)):
                        py = pp.tile([128, 512], _F32, name="py", tag="mm")
                        for mc in range(KC):
                            nc.tensor.matmul(
                                py[0:qsz, 0:w1 - w0],
                                attnT[mc][:, q0:q1],
                                wp_sb[mc][:, w0:w1],
                                start=(mc == 0), stop=(mc == KC - 1),
                            )
                        nc.scalar.copy(ysb[0:qsz, w0:w1], py[0:qsz, 0:w1 - w0])
                    nc.sync.dma_start(y_d[s, q0:q1, :], ysb[0:qsz, :])

            # Software pipeline with decoupled priorities: PSUM slot
            # rotation binds in tile-creation order, engine preference in
            # bass_priority order. Emit front(i+1) BEFORE back(i) so the
            # next sample's projections get PSUM slots during attention,
            # but number back(i) earlier so the scheduler still prefers
            # the critical attention stream when both are ready.
            STRIDE = 4000
            tc.cur_priority = 500
            prev = front(0)
            for i in range(len(samples)):
                if i + 1 < len(samples):
                    tc.cur_priority = STRIDE * i + STRIDE // 2
                    nxt = front(i + 1)
                else:
                    nxt = None
                tc.cur_priority = STRIDE * i + 1000
                back(i, prev)
                prev = nxt
            tc.cur_priority = STRIDE * len(samples)

    _legalize_waits(nc)
    return nc


_NC_CACHE = {}


def _get_module():
    key = (PROJ_DT_NAME, ATT_DT_NAME, PHASES, REPS)
    if key not in _NC_CACHE:
        _NC_CACHE[key] = build_module()
    return _NC_CACHE[key]


def kernel(x, temp_mask, W_qkv, W_proj, b_proj, t_h=None, t_w=None, s_h=None, s_w=None):
    x = np.asarray(x, dtype=np.float32)
    temp_mask = np.asarray(temp_mask, dtype=np.float32)
    B = x.shape[0]
    assert x.shape == (32, N, C), x.shape

    pdt_np = ml_dtypes.bfloat16 if PROJ_DT_NAME == "bfloat16" else np.float32
    wqkvT = np.ascontiguousarray(np.asarray(W_qkv, np.float32).T).astype(pdt_np)
    wpT = np.ascontiguousarray(np.asarray(W_proj, np.float32).T).astype(pdt_np)
    tm = np.ascontiguousarray(temp_mask.reshape(B, 64))

    nc = _get_module()
    per = B // NCORES
    in_maps = [
        {
            "x": np.ascontiguousarray(x[c * per:(c + 1) * per]),
            "tmask": np.ascontiguousarray(tm[c * per:(c + 1) * per]),
            "wqkvT": wqkvT,
            "wpT": wpT,
        }
        for c in range(NCORES)
    ]
    res = run_bass_kernel_spmd(nc, in_maps, core_ids=list(range(NCORES)), trace=TRACE)
    kernel.last_result = res
    y = np.concatenate([res.results[c]["y"] for c in range(NCORES)], axis=0)
    y = y + np.asarray(b_proj, np.float32)[None, None, :]
    return y.astype(np.float32)
